# revision 3
# baseline (speedup 1.0000x reference)
"""CLIP-style contrastive (HCL) loss for B=4096, f32 logits on 8 trn2 cores.

Math reduction (BETA=1, t=0.5, tau+=0.1):
  - imp == neg, so reweight_neg = sum(neg^2) * N / sum(neg).
  - Row i and row i+B of the 2Bx2B sim matrix hold identical value multisets
    (both are {row_i(L), col_i(L)} minus two copies of L[i,i]), so
    loss[i] == loss[i+B] and the mean over 2B rows == mean over B rows.
  - Everything reduces to row sums + col sums of E = exp(2L) and E2 = exp(4L),
    plus the diagonal of L.

Device work per core (rows k*512..(k+1)*512 of L, cast to bf16 on host):
  - row-tiles [128, 4096]; tile 0 split into smaller leading pieces so the
    first EXP starts as soon as the first chunk of DMA lands.
  - ACT: exp(2x)->bf16 E with fused fp32 row-sum (1 elem/cycle/lane, hard
    floor ~13.7us/core). DVE: E2 = E*E via scalar_tensor_tensor with fused
    fp32 row-sum (1x mode, ~1.15ns/elem).
  - Engine balance: DVE total (18.8us) > ACT total (15.4us), so the LAST
    ACT4_COLS columns of the last tile get E2 = exp(4x) directly on ACT
    (same exp table, scale=4.0, fused row-sum) after all exp(2x) pieces.
    This equalizes both engines at ~16.5us.
  - PE: ones-matmul per 128-col block accumulates per-column sums of E and
    E2 into PSUM (chunk-stationary layout [128, 32] each).
Host: assemble sums, per-row loss formula over 4096 rows in f64, mean.
"""

import os

import numpy as np
import ml_dtypes

import concourse.bacc as bacc
import concourse.bass as bass
import concourse.tile as tile
from concourse import mybir
from concourse.bass_utils import run_bass_kernel_spmd

B = 4096
N_CORES = 8
ROWS_PER_CORE = B // N_CORES  # 512
P = 128
TILES = ROWS_PER_CORE // P  # 4

TAU_PLUS = 0.1
TEMPERATURE = 0.5
EPS = 1e-8

# Leading-piece split of tile 0 (cols), so ACT starts early.
T0_SPLIT = [int(x) for x in os.environ.get("KERNEL_T0_SPLIT", "1024,1024,2048").split(",")]
assert sum(T0_SPLIT) == B
# Trailing columns of the LAST tile whose E2 comes from ACT exp(4x) instead
# of DVE square. Multiple of 128.
ACT4_COLS = int(os.environ.get("KERNEL_ACT4_COLS", "2944"))
assert ACT4_COLS % P == 0 and 0 <= ACT4_COLS <= B
LPOOL_BUFS = int(os.environ.get("KERNEL_LPOOL_BUFS", "3"))
EPOOL_BUFS = int(os.environ.get("KERNEL_EPOOL_BUFS", "3"))

# (tile, col_start, col_len) pieces in processing order.
PIECES = []
c0 = 0
for w in T0_SPLIT:
    PIECES.append((0, c0, w))
    c0 += w
for t in range(1, TILES):
    PIECES.append((t, 0, B))
NPIECE = len(PIECES)

_NC = None
LAST_RESULTS = None  # BassKernelResults of the most recent run (for test harness)


def _build_bass():
    in_dt = mybir.dt.bfloat16
    edt = mybir.dt.bfloat16

    nc = bacc.Bacc(None)
    slab = nc.declare_dram_parameter("slab", [ROWS_PER_CORE, B], in_dt, isOutput=False)
    # rs columns: [0, NPIECE) = rowsum_E per piece; [NPIECE, 2*NPIECE) =
    # rowsum_E2 (DVE part) per piece; col 2*NPIECE = rowsum_E2 ACT4 partial.
    NRS = 2 * NPIECE + 1
    rowsums = nc.declare_dram_parameter(
        "rowsums", [P, NRS], mybir.dt.float32, isOutput=True
    )
    # Chunk-stationary layout: [128, 64] (E cols 0:32, E2 cols 32:64), where
    # colsum[m*128 + j] = out[j, m].
    M = B // P  # 32
    colsums = nc.declare_dram_parameter(
        "colsums", [P, 2 * M], mybir.dt.float32, isOutput=True
    )

    with tile.TileContext(nc) as tc:
        with (
            tc.tile_pool(name="lpool", bufs=LPOOL_BUFS) as lpool,
            tc.tile_pool(name="epool", bufs=EPOOL_BUFS) as epool,
            tc.tile_pool(name="e2pool", bufs=EPOOL_BUFS) as e2pool,
            tc.tile_pool(name="singles", bufs=1) as singles,
            tc.tile_pool(name="psum", bufs=1, space="PSUM") as psum_pool,
        ):
            ones = nc.const_aps.tensor(1.0, (P, 1), mybir.dt.bfloat16)
            rs = singles.tile([P, NRS], mybir.dt.float32)
            # One PSUM bank per accumulator; output [128, 32] each.
            psE = psum_pool.tile([P, M], mybir.dt.float32)
            psE2 = psum_pool.tile([P, M], mybir.dt.float32)

            deferred = None  # (ltile, e2tile, dve_cols, tile#) for ACT4 tail
            for i, (t, cs, clen) in enumerate(PIECES):
                rows = slice(t * P, (t + 1) * P)
                cols = slice(cs, cs + clen)
                last_piece = i == NPIECE - 1

                ltile = lpool.tile([P, clen], in_dt, tag="ltile")
                nc.sync.dma_start(out=ltile, in_=slab[rows, cols])

                etile = epool.tile([P, clen], edt, tag="etile")
                nc.scalar.activation(
                    out=etile,
                    in_=ltile,
                    func=mybir.ActivationFunctionType.Exp,
                    scale=2.0,
                    accum_out=rs[:, i : i + 1],
                )
                e2tile = e2pool.tile([P, clen], edt, tag="e2tile")
                dve_cols = clen - ACT4_COLS if last_piece else clen
                if dve_cols > 0:
                    # E2 = (E * 1) * E on DVE, with fused fp32 row-sum.
                    nc.vector.scalar_tensor_tensor(
                        out=e2tile[:, 0:dve_cols],
                        in0=etile[:, 0:dve_cols],
                        scalar=1.0,
                        in1=etile[:, 0:dve_cols],
                        op0=mybir.AluOpType.mult,
                        op1=mybir.AluOpType.mult,
                        accum_out=rs[:, NPIECE + i : NPIECE + i + 1],
                    )

                # PSUM start_tensor_calc zeroes the whole 2KB (partition, bank)
                # zero-region lazily: only the FIRST matmul touching each psum
                # tensor may carry start=True; later writes to still-pending
                # bytes replace (i.e. add to zero), writes to touched bytes
                # accumulate. One start per tensor, ever. The stop for psE2
                # goes on the ACT4 tail blocks emitted after this loop.
                nblk = clen // P
                for m in range(nblk):
                    gm = cs // P + m
                    lsl = slice(m * P, (m + 1) * P)
                    first = i == 0 and m == 0
                    lastE = last_piece and m == nblk - 1
                    nc.tensor.matmul(
                        psE[:, gm : gm + 1],
                        etile[:, lsl],
                        ones,
                        start=first,
                        stop=lastE,
                        skip_group_check=True,
                    )
                    if m * P < dve_cols:
                        lastE2 = last_piece and ACT4_COLS == 0 and m == nblk - 1
                        nc.tensor.matmul(
                            psE2[:, gm : gm + 1],
                            e2tile[:, lsl],
                            ones,
                            start=first,
                            stop=lastE2,
                            skip_group_check=True,
                        )
                if last_piece and ACT4_COLS > 0:
                    deferred = (ltile, e2tile, dve_cols, t, cs)

            if deferred is not None:
                ltile, e2tile, dve_cols, t, cs = deferred
                # E2 tail on ACT: exp(4x) straight from the input, fused row-sum.
                nc.scalar.activation(
                    out=e2tile[:, dve_cols:],
                    in_=ltile[:, dve_cols:],
                    func=mybir.ActivationFunctionType.Exp,
                    scale=4.0,
                    accum_out=rs[:, 2 * NPIECE : 2 * NPIECE + 1],
                )
                for m in range(dve_cols // P, B // P):
                    gm = cs // P + m
                    lsl = slice(m * P, (m + 1) * P)
                    nc.tensor.matmul(
                        psE2[:, gm : gm + 1],
                        e2tile[:, lsl],
                        ones,
                        start=False,
                        stop=m == B // P - 1,
                        skip_group_check=True,
                    )

            # rowsums is ready at the last accum write — issue its DMA before
            # the eviction-gated colsums DMA so it doesn't queue behind it.
            nc.sync.dma_start(out=rowsums[:, :], in_=rs)
            cstile = singles.tile([P, 2 * M], mybir.dt.float32)
            nc.vector.tensor_copy(cstile[:, 0:M], psE)
            nc.scalar.copy(cstile[:, M : 2 * M], psE2)
            nc.sync.dma_start(out=colsums[:, :], in_=cstile)
    # Bacc defers register allocation and sync-wait splitting to finalize();
    # run_bass_via_pjrt does not call it, so do it here.
    nc.finalize()
    return nc


def _get_nc():
    global _NC
    if _NC is None:
        _NC = _build_bass()
    return _NC


def kernel(logits: np.ndarray) -> np.ndarray:
    global LAST_RESULTS
    logits = np.ascontiguousarray(np.asarray(logits, dtype=np.float32))
    assert logits.shape == (B, B)

    nc = _get_nc()
    cast = lambda a: np.ascontiguousarray(a.astype(ml_dtypes.bfloat16))
    in_maps = [
        {"slab": cast(logits[k * ROWS_PER_CORE : (k + 1) * ROWS_PER_CORE, :])}
        for k in range(N_CORES)
    ]
    res = run_bass_kernel_spmd(
        nc,
        in_maps,
        core_ids=list(range(N_CORES)),
        trace=bool(int(os.environ.get("KERNEL_TRACE", "0"))),
    )
    LAST_RESULTS = res

    rowsum_E = np.empty(B, dtype=np.float64)
    rowsum_E2 = np.empty(B, dtype=np.float64)
    colsum_E = np.zeros(B, dtype=np.float64)
    colsum_E2 = np.zeros(B, dtype=np.float64)
    M = B // P
    last_tile = PIECES[-1][0]
    for k in range(N_CORES):
        r = res.results[k]
        rs = r["rowsums"].astype(np.float64)
        sl = slice(k * ROWS_PER_CORE, (k + 1) * ROWS_PER_CORE)
        rsE = np.zeros((P, TILES))
        rsE2 = np.zeros((P, TILES))
        for i, (t, _, clen) in enumerate(PIECES):
            rsE[:, t] += rs[:, i]
            dve_cols = clen - ACT4_COLS if i == NPIECE - 1 else clen
            if dve_cols > 0:
                rsE2[:, t] += rs[:, NPIECE + i]
        if ACT4_COLS > 0:
            rsE2[:, last_tile] += rs[:, 2 * NPIECE]
        rowsum_E[sl] = rsE.T.reshape(-1)
        rowsum_E2[sl] = rsE2.T.reshape(-1)
        cssum = r["colsums"].astype(np.float64)
        colsum_E += cssum[:, :M].T.reshape(-1)
        colsum_E2 += cssum[:, M:].T.reshape(-1)

    d = np.diagonal(logits)
    pos = np.exp(d.astype(np.float64) / TEMPERATURE)
    # The device sums contain exp of the bf16-rounded diagonal; subtract
    # exactly what the device added.
    dD = d.astype(ml_dtypes.bfloat16).astype(np.float64)
    posD = np.exp(dD / TEMPERATURE)
    N = 2 * B - 2
    S1 = rowsum_E + colsum_E - 2.0 * posD
    S2 = rowsum_E2 + colsum_E2 - 2.0 * posD * posD
    reweight = S2 * N / S1
    Ng = (-TAU_PLUS * N * pos + reweight) / (1.0 - TAU_PLUS)
    Ng = np.maximum(Ng, N * np.exp(-1.0 / TEMPERATURE))
    loss = -np.log(pos / (pos + Ng + EPS))
    return np.float32(loss.mean())


# revision 5
# speedup vs baseline: 1.1680x; 1.1680x over previous
"""CLIP-style contrastive (HCL) loss for B=4096, f32 logits on 8 trn2 cores.

Math reduction (BETA=1, t=0.5, tau+=0.1):
  - imp == neg, so reweight_neg = sum(neg^2) * N / sum(neg).
  - Row i and row i+B of the 2Bx2B sim matrix hold identical value multisets
    (both are {row_i(L), col_i(L)} minus two copies of L[i,i]), so
    loss[i] == loss[i+B] and the mean over 2B rows == mean over B rows.
  - Everything reduces to row sums + col sums of E = exp(2L) and E2 = exp(4L),
    plus the diagonal of L.

Device work per core (rows k*512..(k+1)*512 of L, cast to bf16 on host):
  - row-tiles [128, 4096]; tile 0 split into smaller leading pieces so the
    first EXP starts as soon as the first chunk of DMA lands.
  - ACT: exp(2x)->bf16 E with fused fp32 row-sum (1 elem/cycle/lane, hard
    floor ~13.7us/core). DVE: E2 = E*E via scalar_tensor_tensor with fused
    fp32 row-sum (1x mode, ~1.15ns/elem).
  - Engine balance: DVE total (18.8us) > ACT total (15.4us), so the LAST
    ACT4_COLS columns of the last tile get E2 = exp(4x) directly on ACT
    (same exp table, scale=4.0, fused row-sum) after all exp(2x) pieces.
    This equalizes both engines at ~16.5us.
  - PE: ones-matmul per 128-col block accumulates per-column sums of E and
    E2 into PSUM (chunk-stationary layout [128, 32] each).
Host: assemble sums, per-row loss formula over 4096 rows in f64, mean.
"""

import os

import numpy as np
import ml_dtypes

import concourse.bacc as bacc
import concourse.bass as bass
import concourse.tile as tile
from concourse import mybir
from concourse.bass_utils import run_bass_kernel_spmd

B = 4096
N_CORES = 8
ROWS_PER_CORE = B // N_CORES  # 512
P = 128
TILES = ROWS_PER_CORE // P  # 4

TAU_PLUS = 0.1
TEMPERATURE = 0.5
EPS = 1e-8

# Leading-piece split of tile 0 (cols), so ACT starts early. Ops wider than
# ~2048 run ~13% slower per element (SBUF contention), so cap at 2048.
T0_SPLIT = [int(x) for x in os.environ.get("KERNEL_T0_SPLIT", "1024,1024,2048").split(",")]
assert sum(T0_SPLIT) == B
TN_SPLIT = [int(x) for x in os.environ.get("KERNEL_TN_SPLIT", "2048,2048").split(",")]
assert sum(TN_SPLIT) == B
# Trailing columns of the LAST piece whose E2 comes from ACT exp(4x) instead
# of DVE square (engine balancing: DVE total > ACT total). Multiple of 128.
ACT4_COLS = int(os.environ.get("KERNEL_ACT4_COLS", "1536"))
LPOOL_BUFS = int(os.environ.get("KERNEL_LPOOL_BUFS", "4"))
EPOOL_BUFS = int(os.environ.get("KERNEL_EPOOL_BUFS", "3"))

# (tile, col_start, col_len) pieces in processing order.
PIECES = []
for t in range(TILES):
    c0 = 0
    for w in T0_SPLIT if t == 0 else TN_SPLIT:
        PIECES.append((t, c0, w))
        c0 += w
NPIECE = len(PIECES)
assert ACT4_COLS % P == 0 and 0 <= ACT4_COLS <= PIECES[-1][2]

_NC = None
LAST_RESULTS = None  # BassKernelResults of the most recent run (for test harness)


def _build_bass():
    in_dt = mybir.dt.bfloat16
    edt = mybir.dt.bfloat16

    nc = bacc.Bacc(None)
    slab = nc.declare_dram_parameter("slab", [ROWS_PER_CORE, B], in_dt, isOutput=False)
    # rs columns: [0, NPIECE) = rowsum_E per piece; [NPIECE, 2*NPIECE) =
    # rowsum_E2 (DVE part) per piece; col 2*NPIECE = rowsum_E2 ACT4 partial.
    NRS = 2 * NPIECE + 1
    rowsums = nc.declare_dram_parameter(
        "rowsums", [P, NRS], mybir.dt.float32, isOutput=True
    )
    # Chunk-stationary layout: [128, 64] (E cols 0:32, E2 cols 32:64), where
    # colsum[m*128 + j] = out[j, m].
    M = B // P  # 32
    colsums = nc.declare_dram_parameter(
        "colsums", [P, 2 * M], mybir.dt.float32, isOutput=True
    )

    with tile.TileContext(nc) as tc:
        with (
            tc.tile_pool(name="lpool", bufs=LPOOL_BUFS) as lpool,
            tc.tile_pool(name="epool", bufs=EPOOL_BUFS) as epool,
            tc.tile_pool(name="e2pool", bufs=EPOOL_BUFS) as e2pool,
            tc.tile_pool(name="singles", bufs=1) as singles,
            tc.tile_pool(name="psum", bufs=1, space="PSUM") as psum_pool,
        ):
            ones = nc.const_aps.tensor(1.0, (P, 1), mybir.dt.bfloat16)
            rs = singles.tile([P, NRS], mybir.dt.float32)
            # One PSUM bank per accumulator; output [128, 32] each.
            psE = psum_pool.tile([P, M], mybir.dt.float32)
            psE2 = psum_pool.tile([P, M], mybir.dt.float32)

            deferred = None  # (ltile, e2tile, dve_cols, tile#) for ACT4 tail
            for i, (t, cs, clen) in enumerate(PIECES):
                rows = slice(t * P, (t + 1) * P)
                cols = slice(cs, cs + clen)
                last_piece = i == NPIECE - 1

                ltile = lpool.tile([P, clen], in_dt, tag="ltile")
                nc.sync.dma_start(out=ltile, in_=slab[rows, cols])

                etile = epool.tile([P, clen], edt, tag="etile")
                nc.scalar.activation(
                    out=etile,
                    in_=ltile,
                    func=mybir.ActivationFunctionType.Exp,
                    scale=2.0,
                    accum_out=rs[:, i : i + 1],
                )
                e2tile = e2pool.tile([P, clen], edt, tag="e2tile")
                dve_cols = clen - ACT4_COLS if last_piece else clen
                if dve_cols > 0:
                    # E2 = (E * 1) * E on DVE, with fused fp32 row-sum.
                    nc.vector.scalar_tensor_tensor(
                        out=e2tile[:, 0:dve_cols],
                        in0=etile[:, 0:dve_cols],
                        scalar=1.0,
                        in1=etile[:, 0:dve_cols],
                        op0=mybir.AluOpType.mult,
                        op1=mybir.AluOpType.mult,
                        accum_out=rs[:, NPIECE + i : NPIECE + i + 1],
                    )

                # PSUM start_tensor_calc zeroes the whole 2KB (partition, bank)
                # zero-region lazily: only the FIRST matmul touching each psum
                # tensor may carry start=True; later writes to still-pending
                # bytes replace (i.e. add to zero), writes to touched bytes
                # accumulate. One start per tensor, ever. The stop for psE2
                # goes on the ACT4 tail blocks emitted after this loop.
                nblk = clen // P
                for m in range(nblk):
                    gm = cs // P + m
                    lsl = slice(m * P, (m + 1) * P)
                    first = i == 0 and m == 0
                    lastE = last_piece and m == nblk - 1
                    nc.tensor.matmul(
                        psE[:, gm : gm + 1],
                        etile[:, lsl],
                        ones,
                        start=first,
                        stop=lastE,
                        skip_group_check=True,
                    )
                    if m * P < dve_cols:
                        lastE2 = last_piece and ACT4_COLS == 0 and m == nblk - 1
                        nc.tensor.matmul(
                            psE2[:, gm : gm + 1],
                            e2tile[:, lsl],
                            ones,
                            start=first,
                            stop=lastE2,
                            skip_group_check=True,
                        )
                if last_piece and ACT4_COLS > 0:
                    deferred = (ltile, e2tile, dve_cols, t, cs)

            if deferred is not None:
                ltile, e2tile, dve_cols, t, cs = deferred
                # E2 tail on ACT: exp(4x) straight from the input, fused row-sum.
                nc.scalar.activation(
                    out=e2tile[:, dve_cols:],
                    in_=ltile[:, dve_cols:],
                    func=mybir.ActivationFunctionType.Exp,
                    scale=4.0,
                    accum_out=rs[:, 2 * NPIECE : 2 * NPIECE + 1],
                )
                clen = PIECES[-1][2]
                for m in range(dve_cols // P, clen // P):
                    gm = cs // P + m
                    lsl = slice(m * P, (m + 1) * P)
                    nc.tensor.matmul(
                        psE2[:, gm : gm + 1],
                        e2tile[:, lsl],
                        ones,
                        start=False,
                        stop=m == clen // P - 1,
                        skip_group_check=True,
                    )

            # rowsums is ready at the last accum write — issue its DMA before
            # the eviction-gated colsums DMA so it doesn't queue behind it.
            nc.sync.dma_start(out=rowsums[:, :], in_=rs)
            cstile = singles.tile([P, 2 * M], mybir.dt.float32)
            nc.vector.tensor_copy(cstile[:, 0:M], psE)
            nc.scalar.copy(cstile[:, M : 2 * M], psE2)
            nc.sync.dma_start(out=colsums[:, :], in_=cstile)
    # Bacc defers register allocation and sync-wait splitting to finalize();
    # run_bass_via_pjrt does not call it, so do it here.
    nc.finalize()
    return nc


def _get_nc():
    global _NC
    if _NC is None:
        _NC = _build_bass()
    return _NC


def kernel(logits: np.ndarray) -> np.ndarray:
    global LAST_RESULTS
    logits = np.ascontiguousarray(np.asarray(logits, dtype=np.float32))
    assert logits.shape == (B, B)

    nc = _get_nc()
    cast = lambda a: np.ascontiguousarray(a.astype(ml_dtypes.bfloat16))
    in_maps = [
        {"slab": cast(logits[k * ROWS_PER_CORE : (k + 1) * ROWS_PER_CORE, :])}
        for k in range(N_CORES)
    ]
    res = run_bass_kernel_spmd(
        nc,
        in_maps,
        core_ids=list(range(N_CORES)),
        trace=bool(int(os.environ.get("KERNEL_TRACE", "0"))),
    )
    LAST_RESULTS = res

    rowsum_E = np.empty(B, dtype=np.float64)
    rowsum_E2 = np.empty(B, dtype=np.float64)
    colsum_E = np.zeros(B, dtype=np.float64)
    colsum_E2 = np.zeros(B, dtype=np.float64)
    M = B // P
    last_tile = PIECES[-1][0]
    for k in range(N_CORES):
        r = res.results[k]
        rs = r["rowsums"].astype(np.float64)
        sl = slice(k * ROWS_PER_CORE, (k + 1) * ROWS_PER_CORE)
        rsE = np.zeros((P, TILES))
        rsE2 = np.zeros((P, TILES))
        for i, (t, _, clen) in enumerate(PIECES):
            rsE[:, t] += rs[:, i]
            dve_cols = clen - ACT4_COLS if i == NPIECE - 1 else clen
            if dve_cols > 0:
                rsE2[:, t] += rs[:, NPIECE + i]
        if ACT4_COLS > 0:
            rsE2[:, last_tile] += rs[:, 2 * NPIECE]
        rowsum_E[sl] = rsE.T.reshape(-1)
        rowsum_E2[sl] = rsE2.T.reshape(-1)
        cssum = r["colsums"].astype(np.float64)
        colsum_E += cssum[:, :M].T.reshape(-1)
        colsum_E2 += cssum[:, M:].T.reshape(-1)

    d = np.diagonal(logits)
    pos = np.exp(d.astype(np.float64) / TEMPERATURE)
    # The device sums contain exp of the bf16-rounded diagonal; subtract
    # exactly what the device added.
    dD = d.astype(ml_dtypes.bfloat16).astype(np.float64)
    posD = np.exp(dD / TEMPERATURE)
    N = 2 * B - 2
    S1 = rowsum_E + colsum_E - 2.0 * posD
    S2 = rowsum_E2 + colsum_E2 - 2.0 * posD * posD
    reweight = S2 * N / S1
    Ng = (-TAU_PLUS * N * pos + reweight) / (1.0 - TAU_PLUS)
    Ng = np.maximum(Ng, N * np.exp(-1.0 / TEMPERATURE))
    loss = -np.log(pos / (pos + Ng + EPS))
    return np.float32(loss.mean())


# revision 6
# speedup vs baseline: 1.1896x; 1.0185x over previous
"""CLIP-style contrastive (HCL) loss for B=4096, f32 logits on 8 trn2 cores.

Math reduction (BETA=1, t=0.5, tau+=0.1):
  - imp == neg, so reweight_neg = sum(neg^2) * N / sum(neg).
  - Row i and row i+B of the 2Bx2B sim matrix hold identical value multisets
    (both are {row_i(L), col_i(L)} minus two copies of L[i,i]), so
    loss[i] == loss[i+B] and the mean over 2B rows == mean over B rows.
  - Everything reduces to row sums + col sums of E = exp(2L) and E2 = exp(4L),
    plus the diagonal of L.

Device work per core (rows k*512..(k+1)*512 of L, cast to bf16 on host):
  - row-tiles [128, 4096]; tile 0 split into smaller leading pieces so the
    first EXP starts as soon as the first chunk of DMA lands.
  - ACT: exp(2x)->bf16 E with fused fp32 row-sum (1 elem/cycle/lane, hard
    floor ~13.7us/core). DVE: E2 = E*E via scalar_tensor_tensor with fused
    fp32 row-sum (1x mode, ~1.15ns/elem).
  - Engine balance: DVE total (18.8us) > ACT total (15.4us), so the LAST
    ACT4_COLS columns of the last tile get E2 = exp(4x) directly on ACT
    (same exp table, scale=4.0, fused row-sum) after all exp(2x) pieces.
    This equalizes both engines at ~16.5us.
  - PE: ones-matmul per 128-col block accumulates per-column sums of E and
    E2 into PSUM (chunk-stationary layout [128, 32] each).
Host: assemble sums, per-row loss formula over 4096 rows in f64, mean.
"""

import os

import numpy as np
import ml_dtypes

import concourse.bacc as bacc
import concourse.bass as bass
import concourse.tile as tile
from concourse import mybir
from concourse.bass_utils import run_bass_kernel_spmd

B = 4096
N_CORES = 8
ROWS_PER_CORE = B // N_CORES  # 512
P = 128
TILES = ROWS_PER_CORE // P  # 4

TAU_PLUS = 0.1
TEMPERATURE = 0.5
EPS = 1e-8

# Leading-piece split of tile 0 (cols), so ACT starts early. Ops wider than
# ~2048 run ~13% slower per element (SBUF contention), so cap at 2048.
T0_SPLIT = [int(x) for x in os.environ.get("KERNEL_T0_SPLIT", "1024,1024,2048").split(",")]
assert sum(T0_SPLIT) == B
TN_SPLIT = [int(x) for x in os.environ.get("KERNEL_TN_SPLIT", "2048,2048").split(",")]
assert sum(TN_SPLIT) == B
# Trailing columns of the LAST piece whose E2 comes from ACT exp(4x) instead
# of DVE square (engine balancing: DVE total > ACT total). Multiple of 128.
ACT4_COLS = int(os.environ.get("KERNEL_ACT4_COLS", "1536"))
LPOOL_BUFS = int(os.environ.get("KERNEL_LPOOL_BUFS", "6"))
EPOOL_BUFS = int(os.environ.get("KERNEL_EPOOL_BUFS", "4"))

# (tile, col_start, col_len) pieces in processing order.
PIECES = []
for t in range(TILES):
    c0 = 0
    for w in T0_SPLIT if t == 0 else TN_SPLIT:
        PIECES.append((t, c0, w))
        c0 += w
NPIECE = len(PIECES)
assert ACT4_COLS % P == 0 and 0 <= ACT4_COLS <= PIECES[-1][2]

_NC = None
LAST_RESULTS = None  # BassKernelResults of the most recent run (for test harness)


def _build_bass():
    in_dt = mybir.dt.bfloat16
    edt = mybir.dt.bfloat16

    nc = bacc.Bacc(None)
    slab = nc.declare_dram_parameter("slab", [ROWS_PER_CORE, B], in_dt, isOutput=False)
    # rs columns: [0, NPIECE) = rowsum_E per piece; [NPIECE, 2*NPIECE) =
    # rowsum_E2 (DVE part) per piece; col 2*NPIECE = rowsum_E2 ACT4 partial.
    NRS = 2 * NPIECE + 1
    rowsums = nc.declare_dram_parameter(
        "rowsums", [P, NRS], mybir.dt.float32, isOutput=True
    )
    # Chunk-stationary layout: [128, 64] (E cols 0:32, E2 cols 32:64), where
    # colsum[m*128 + j] = out[j, m].
    M = B // P  # 32
    colsums = nc.declare_dram_parameter(
        "colsums", [P, 2 * M], mybir.dt.float32, isOutput=True
    )

    with tile.TileContext(nc) as tc:
        with (
            tc.tile_pool(name="lpool", bufs=LPOOL_BUFS) as lpool,
            tc.tile_pool(name="epool", bufs=EPOOL_BUFS) as epool,
            tc.tile_pool(name="e2pool", bufs=EPOOL_BUFS) as e2pool,
            tc.tile_pool(name="singles", bufs=1) as singles,
            tc.tile_pool(name="psum", bufs=1, space="PSUM") as psum_pool,
        ):
            ones = nc.const_aps.tensor(1.0, (P, 1), mybir.dt.bfloat16)
            rs = singles.tile([P, NRS], mybir.dt.float32)
            # One PSUM bank per accumulator; output [128, 32] each.
            psE = psum_pool.tile([P, M], mybir.dt.float32)
            psE2 = psum_pool.tile([P, M], mybir.dt.float32)

            deferred = None  # (ltile, e2tile, dve_cols, tile#) for ACT4 tail
            for i, (t, cs, clen) in enumerate(PIECES):
                rows = slice(t * P, (t + 1) * P)
                cols = slice(cs, cs + clen)
                last_piece = i == NPIECE - 1

                ltile = lpool.tile([P, clen], in_dt, tag="ltile")
                nc.sync.dma_start(out=ltile, in_=slab[rows, cols])

                etile = epool.tile([P, clen], edt, tag="etile")
                nc.scalar.activation(
                    out=etile,
                    in_=ltile,
                    func=mybir.ActivationFunctionType.Exp,
                    scale=2.0,
                    accum_out=rs[:, i : i + 1],
                )
                e2tile = e2pool.tile([P, clen], edt, tag="e2tile")
                dve_cols = clen - ACT4_COLS if last_piece else clen
                if dve_cols > 0:
                    # E2 = (E * 1) * E on DVE, with fused fp32 row-sum.
                    nc.vector.scalar_tensor_tensor(
                        out=e2tile[:, 0:dve_cols],
                        in0=etile[:, 0:dve_cols],
                        scalar=1.0,
                        in1=etile[:, 0:dve_cols],
                        op0=mybir.AluOpType.mult,
                        op1=mybir.AluOpType.mult,
                        accum_out=rs[:, NPIECE + i : NPIECE + i + 1],
                    )

                # PSUM start_tensor_calc zeroes the whole 2KB (partition, bank)
                # zero-region lazily: only the FIRST matmul touching each psum
                # tensor may carry start=True; later writes to still-pending
                # bytes replace (i.e. add to zero), writes to touched bytes
                # accumulate. One start per tensor, ever. The stop for psE2
                # goes on the ACT4 tail blocks emitted after this loop.
                nblk = clen // P
                for m in range(nblk):
                    gm = cs // P + m
                    lsl = slice(m * P, (m + 1) * P)
                    first = i == 0 and m == 0
                    lastE = last_piece and m == nblk - 1
                    nc.tensor.matmul(
                        psE[:, gm : gm + 1],
                        etile[:, lsl],
                        ones,
                        start=first,
                        stop=lastE,
                        skip_group_check=True,
                    )
                    if m * P < dve_cols:
                        lastE2 = last_piece and ACT4_COLS == 0 and m == nblk - 1
                        nc.tensor.matmul(
                            psE2[:, gm : gm + 1],
                            e2tile[:, lsl],
                            ones,
                            start=first,
                            stop=lastE2,
                            skip_group_check=True,
                        )
                if last_piece and ACT4_COLS > 0:
                    deferred = (ltile, e2tile, dve_cols, t, cs)

            if deferred is not None:
                ltile, e2tile, dve_cols, t, cs = deferred
                # E2 tail on ACT: exp(4x) straight from the input, fused row-sum.
                nc.scalar.activation(
                    out=e2tile[:, dve_cols:],
                    in_=ltile[:, dve_cols:],
                    func=mybir.ActivationFunctionType.Exp,
                    scale=4.0,
                    accum_out=rs[:, 2 * NPIECE : 2 * NPIECE + 1],
                )
                clen = PIECES[-1][2]
                for m in range(dve_cols // P, clen // P):
                    gm = cs // P + m
                    lsl = slice(m * P, (m + 1) * P)
                    nc.tensor.matmul(
                        psE2[:, gm : gm + 1],
                        e2tile[:, lsl],
                        ones,
                        start=False,
                        stop=m == clen // P - 1,
                        skip_group_check=True,
                    )

            # rowsums is ready at the last accum write — issue its DMA before
            # the eviction-gated colsums DMA so it doesn't queue behind it.
            nc.sync.dma_start(out=rowsums[:, :], in_=rs)
            cstile = singles.tile([P, 2 * M], mybir.dt.float32)
            nc.vector.tensor_copy(cstile[:, 0:M], psE)
            nc.scalar.copy(cstile[:, M : 2 * M], psE2)
            nc.sync.dma_start(out=colsums[:, :], in_=cstile)
    # Bacc defers register allocation and sync-wait splitting to finalize();
    # run_bass_via_pjrt does not call it, so do it here.
    nc.finalize()
    return nc


def _get_nc():
    global _NC
    if _NC is None:
        _NC = _build_bass()
    return _NC


def kernel(logits: np.ndarray) -> np.ndarray:
    global LAST_RESULTS
    logits = np.ascontiguousarray(np.asarray(logits, dtype=np.float32))
    assert logits.shape == (B, B)

    nc = _get_nc()
    cast = lambda a: np.ascontiguousarray(a.astype(ml_dtypes.bfloat16))
    in_maps = [
        {"slab": cast(logits[k * ROWS_PER_CORE : (k + 1) * ROWS_PER_CORE, :])}
        for k in range(N_CORES)
    ]
    res = run_bass_kernel_spmd(
        nc,
        in_maps,
        core_ids=list(range(N_CORES)),
        trace=bool(int(os.environ.get("KERNEL_TRACE", "0"))),
    )
    LAST_RESULTS = res

    rowsum_E = np.empty(B, dtype=np.float64)
    rowsum_E2 = np.empty(B, dtype=np.float64)
    colsum_E = np.zeros(B, dtype=np.float64)
    colsum_E2 = np.zeros(B, dtype=np.float64)
    M = B // P
    last_tile = PIECES[-1][0]
    for k in range(N_CORES):
        r = res.results[k]
        rs = r["rowsums"].astype(np.float64)
        sl = slice(k * ROWS_PER_CORE, (k + 1) * ROWS_PER_CORE)
        rsE = np.zeros((P, TILES))
        rsE2 = np.zeros((P, TILES))
        for i, (t, _, clen) in enumerate(PIECES):
            rsE[:, t] += rs[:, i]
            dve_cols = clen - ACT4_COLS if i == NPIECE - 1 else clen
            if dve_cols > 0:
                rsE2[:, t] += rs[:, NPIECE + i]
        if ACT4_COLS > 0:
            rsE2[:, last_tile] += rs[:, 2 * NPIECE]
        rowsum_E[sl] = rsE.T.reshape(-1)
        rowsum_E2[sl] = rsE2.T.reshape(-1)
        cssum = r["colsums"].astype(np.float64)
        colsum_E += cssum[:, :M].T.reshape(-1)
        colsum_E2 += cssum[:, M:].T.reshape(-1)

    d = np.diagonal(logits)
    pos = np.exp(d.astype(np.float64) / TEMPERATURE)
    # The device sums contain exp of the bf16-rounded diagonal; subtract
    # exactly what the device added.
    dD = d.astype(ml_dtypes.bfloat16).astype(np.float64)
    posD = np.exp(dD / TEMPERATURE)
    N = 2 * B - 2
    S1 = rowsum_E + colsum_E - 2.0 * posD
    S2 = rowsum_E2 + colsum_E2 - 2.0 * posD * posD
    reweight = S2 * N / S1
    Ng = (-TAU_PLUS * N * pos + reweight) / (1.0 - TAU_PLUS)
    Ng = np.maximum(Ng, N * np.exp(-1.0 / TEMPERATURE))
    loss = -np.log(pos / (pos + Ng + EPS))
    return np.float32(loss.mean())
